# revision 23
# baseline (speedup 1.0000x reference)
"""Expert-parallel top-2 MoE kernel for 8 Trainium2 NeuronCores.

Strategy (v2 — dense-Tensor restructure of the expert-parallel baseline):
  - Router: every core computes fp32 logits for all T tokens locally
    (Wg stationary, xT streamed in 8 chunks), transposes to token-partition
    layout, softmax/top-2 on-device.  No collective on the critical path.
  - Core c owns expert c.  Slot positions via matmul-based exclusive
    cumsums; slot->token map built with one-hot matmuls per group; routed
    token rows indirect-gathered from bf16 x and transposed into xgT.
  - FFN is PHASE-SPLIT per group to keep the TensorE stream dense (the
    HAM clock gate needs continuous execution to stay at 2.4 GHz):
      phase A: all 32 f-tiles of layer 1 -> ReLU -> hbuf (SBUF resident)
      phase B: output-stationary layer 2 (per (slot-tile, D-half): sweep
               all f, one PSUM bank at a time), bias via rank-1 bf16
               matmul, staged out + AllGathered per group.
    Groups are sized [3,3,2,1] tiles so the LAST AllGather (the only
    exposed one) is small.
  - Dummy identity matmuls warm the PE clock before the router and
    before the first FFN group.
  - Combine: per chunk one batched indirect gather of all (q, own-tile)
    rows + gate-weighted fp32 accumulation, overlapped with the next
    group's FFN.  Each core returns its own 512-token shard.

Numerics: router fp32 (top-2 selection fidelity), FFN bf16 with fp32
accumulation in PSUM, combine in fp32.
"""

import os
import sys

import numpy as np

for _p in ("/opt/trn_rl_repo",):
    if _p not in sys.path:
        sys.path.append(_p)

import ml_dtypes

import concourse.bass as bass
import concourse.mybir as mybir
import concourse.tile as tile
from concourse import bacc
from concourse.bass import IndirectOffsetOnAxis
from concourse.masks import make_identity

# Problem shapes (fixed per spec)
B, S, D, E = 2, 2048, 1024, 8
T = B * S          # 4096 tokens
F = 4 * D          # 4096 ffn dim
P = 128            # partitions
NT = T // P        # 32 token tiles
KD = D // P        # 8 contraction tiles over D
NF = F // P        # 32 f tiles
NE = NT * E        # 256
TOK_PER_CORE = T // E   # 512
OWN_TILES = TOK_PER_CORE // P  # 4
N_CORES = E
BIGP = 100000.0    # OOB slot id for tokens not routed here

f32 = mybir.dt.float32
bf16 = mybir.dt.bfloat16
i32 = mybir.dt.int32
u32 = mybir.dt.uint32

_cache = {}


def _group_tiles(st: int) -> list[int]:
    """Split ST slot tiles into FFN/AG groups; biggest first, small last."""
    gs = [1]
    rem = st - 1
    while rem > 4:
        gs.append(3)
        rem -= 3
    if rem == 4:
        gs += [3, 1]
    elif rem == 3:
        gs += [2, 1]
    elif rem > 0:
        gs += [rem]
    return gs


def build_module(ST: int):
    """Build the SPMD Bass module for capacity C = 128*ST."""
    C = P * ST
    nc = bacc.Bacc("TRN2", target_bir_lowering=False, debug=False,
                   num_devices=N_CORES)

    # ---- I/O ----
    xTf = nc.dram_tensor("xTf", [D, T], f32, kind="ExternalInput").ap()
    xbf = nc.dram_tensor("xbf", [T, D], bf16, kind="ExternalInput").ap()
    w1d = nc.dram_tensor("w1d", [NF * P, KD * P], bf16,
                         kind="ExternalInput").ap()
    w2d = nc.dram_tensor("w2d", [F, D], bf16, kind="ExternalInput").ap()
    wgd = nc.dram_tensor("wgd", [D, E], f32, kind="ExternalInput").ap()
    bgb = nc.dram_tensor("bgb", [P, NE], f32, kind="ExternalInput").ap()
    b1pm = nc.dram_tensor("b1pm", [P, NF], f32, kind="ExternalInput").ap()
    b2bd = nc.dram_tensor("b2bd", [1, D], bf16, kind="ExternalInput").ap()
    sel256 = nc.dram_tensor("sel256", [P, NE], f32,
                            kind="ExternalInput").ap()
    selEd = nc.dram_tensor("selEd", [E, P], f32, kind="ExternalInput").ap()
    l128d = nc.dram_tensor("l128d", [P, P], f32, kind="ExternalInput").ap()
    ownmd = nc.dram_tensor("ownmd", [P, OWN_TILES * NT], f32,
                           kind="ExternalInput").ap()
    out = nc.dram_tensor("out", [TOK_PER_CORE, D], bf16,
                         kind="ExternalOutput").ap()

    with tile.TileContext(nc) as tc:
        _emit(tc, ST, xTf, xbf, w1d, w2d, wgd, bgb, b1pm, b2bd, sel256,
              selEd, l128d, ownmd, out)

    nc.compile()
    return nc


def _emit(tc, ST, xTf, xbf, w1d, w2d, wgd, bgb, b1pm, b2bd, sel256,
          selEd, l128d, ownmd, out):
    nc = tc.nc
    C = P * ST
    gtiles = _group_tiles(ST)            # tiles per group
    NG = len(gtiles)
    gsz = [P * t for t in gtiles]        # slots per group
    gbase = [P * sum(gtiles[:i]) for i in range(NG)]  # slot base per group
    NSEL = 2 * NG                        # selection planes (g, q)

    # ---------------- persistent pools ----------------
    persist = tc.alloc_tile_pool(name="persist", bufs=1)
    dram = tc.alloc_tile_pool(name="dram", bufs=1, space="DRAM")

    # tiny warmup AllGather: absorbs first-collective setup cost
    wup_in = dram.tile([E, 4], f32, name="wup_in")
    wup_out = dram.tile([N_CORES * E, 4], f32, addr_space="Shared",
                        name="wup_out")
    nc.gpsimd.collective_compute(
        "AllGather", mybir.AluOpType.bypass,
        replica_groups=[list(range(N_CORES))],
        ins=[wup_in[:].opt()], outs=[wup_out[:].opt()],
    )

    # router inputs first (split across DMA queues so the router starts fast)
    wg_sb = persist.tile([P, KD, E], f32, name="wg_sb")
    nc.sync.dma_start(wg_sb[:], wgd.rearrange("(k p) e -> p k e", p=P))
    ownm_sb = persist.tile([P, OWN_TILES * NT], f32, name="ownm_sb")
    nc.sync.dma_start(ownm_sb[:], ownmd[:])
    sel_sb = persist.tile([P, NE], f32, name="sel_sb")
    nc.sync.dma_start(sel_sb[:], sel256[:])
    selE_sb = persist.tile([E, P], f32, name="selE_sb")
    nc.sync.dma_start(selE_sb[:], selEd[:])
    l128_sb = persist.tile([P, P], f32, name="l128_sb")
    nc.sync.dma_start(l128_sb[:], l128d[:])
    b1_sb = persist.tile([P, NF], f32, name="b1_sb")
    nc.sync.dma_start(b1_sb[:], b1pm[:])
    b2_sb = persist.tile([1, D], bf16, name="b2_sb")
    nc.sync.dma_start(b2_sb[:], b2bd[:])
    ident = persist.tile([P, P], bf16, name="ident")
    make_identity(nc, ident[:])
    identf = persist.tile([P, P], f32, name="identf")
    make_identity(nc, identf[:])
    ones_col = persist.tile([P, 1], f32, name="ones_col")
    nc.vector.memset(ones_col[:], 1.0)
    ones_row = persist.tile([1, P], f32, name="ones_row")
    nc.vector.memset(ones_row[:], 1.0)
    ones_rb = persist.tile([1, P], bf16, name="ones_rb")
    nc.vector.memset(ones_rb[:], 1.0)
    wrmi = persist.tile([P, 1], i32, name="wrmi")
    nc.vector.memset(wrmi[:], 0)
    # slot iota (u16: 2x DVE rate) and the [p, tile] pair for idx matmuls
    iotaC = persist.tile([P, C], mybir.dt.uint16, name="iotaC")
    pv2 = persist.tile([P, NT, 2], bf16, name="pv2")

    # state kept across phases
    exp_all = persist.tile([P, NE], f32, name="exp_all")    # exp(logits)
    m8_all = persist.tile([P, NE], f32, name="m8_all")      # per-tile top8
    r_all = persist.tile([P, NT], f32, name="r_all")        # 1/sum(exp)
    pos_all = persist.tile([P, NE], f32, name="pos_all")    # excl cumsum
    ind_all = persist.tile([P, NE], f32, name="ind_all")    # top2 indicator
    ei_all = persist.tile([P, NE], u32, name="ei_all")      # top8 indices
    # combine selection: per own-tile j, plane k = 2*g+q (offsets then gates)
    red_sb = persist.tile([P, OWN_TILES, 2 * NSEL], f32, name="red_sb")
    # batched-gather offsets: per chunk g, columns (q, j)
    redi_sb = persist.tile([P, NG, 2 * OWN_TILES], i32, name="redi_sb")
    ot = [persist.tile([P, D], bf16, name=f"ot{j}")
          for j in range(OWN_TILES)]

    idx_i = persist.tile([P, ST], i32, name="idx_i")
    pos_m = persist.tile([P, NT], f32, name="pos_m")    # my-expert slots
    e1f = persist.tile([P, NT], bf16, name="e1f")       # top-1 expert id
    e2f = persist.tile([P, NT], bf16, name="e2f")       # top-2 expert id

    w1_sb = [persist.tile([P, KD * P], bf16, name=f"w1_sb{f}")
             for f in range(NF)]
    w2_sb = [persist.tile([P, D], bf16, name=f"w2_sb{f}") for f in range(NF)]

    y_dram = [dram.tile([gsz[g], D], bf16, name=f"y_dram{g}")
              for g in range(NG)]
    y_all = [dram.tile([N_CORES * gsz[g], D], bf16, addr_space="Shared",
                       name=f"y_all{g}") for g in range(NG)]

    # ---------------- PE warmup (HAM clock gate) ----------------
    def warm_mms(pool, psum_pool, n, tag):
        pt = psum_pool.tile([P, P], f32, tag=tag, bufs=2, name=tag)
        for i in range(n):
            nc.tensor.matmul(pt[:], lhsT=ident[:], rhs=ident[:],
                             start=True, stop=True, skip_group_check=True)

    # ------------- router: every core computes ALL logits locally -------------
    with tc.tile_pool(name="router_sb", bufs=1, named_scope="router") as rpool, \
         tc.tile_pool(name="router_ps", bufs=1, space="PSUM") as rps:
        bg_sb = rpool.tile([P, NE], f32, name="bg_sb")
        nc.sync.dma_start(bg_sb[:], bgb[:])
        # warmups: ACT table + dynamic-DMA path cold-start absorbers
        wrm = rpool.tile([P, 8], f32, name="wrm")
        nc.scalar.activation(wrm[:], ones_col[:].to_broadcast([P, 8]),
                             mybir.ActivationFunctionType.Relu)
        wrg = rpool.tile([P, D], bf16, name="wrg")
        nc.gpsimd.indirect_dma_start(
            out=wrg[:], out_offset=None, in_=xbf[:],
            in_offset=IndirectOffsetOnAxis(ap=wrmi[:], axis=0))
        iotaC_i = rpool.tile([P, C], i32, name="iotaC_i")
        nc.gpsimd.iota(iotaC_i[:], pattern=[[1, C]], base=0,
                       channel_multiplier=0)
        nc.vector.tensor_copy(iotaC[:], iotaC_i[:])
        pv2_i = rpool.tile([P, NT, 2], i32, name="pv2_i")
        nc.gpsimd.iota(pv2_i[:, :, 0], pattern=[[0, NT]], base=0,
                       channel_multiplier=1)
        nc.gpsimd.iota(pv2_i[:, :, 1], pattern=[[1, NT]], base=0,
                       channel_multiplier=0)
        nc.vector.tensor_copy(pv2[:], pv2_i[:])
        warm_mms(rpool, rps, 64, "wmm0")
        l_all = rpool.tile([P, NE], f32, name="l_all")
        xTv = xTf.rearrange("(k p) t -> p k t", p=P)
        NCH = T // TOK_PER_CORE  # 8 chunks of 512 tokens
        QT = TOK_PER_CORE // P   # 4 token tiles per chunk
        LAG = 2
        lsbs = [None] * NCH

        def emit_transpose(cc):
            pt_l = rps.tile([P, QT, E], f32, tag="pt_l", bufs=2, name="pt_l")
            for q in range(QT):
                nc.tensor.transpose(
                    pt_l[:, q, :], lsbs[cc][:, q * P:(q + 1) * P],
                    identf[:E, :E])
            nc.vector.tensor_copy(
                l_all[:, cc * QT * E:(cc + 1) * QT * E], pt_l[:])

        for c in range(NCH):
            xsc = rpool.tile([P, KD, TOK_PER_CORE], f32, tag="xsc", bufs=2,
                             name="xsc")
            for k in range(KD):
                nc.sync.dma_start(
                    xsc[:, k, :],
                    xTv[:, k, c * TOK_PER_CORE:(c + 1) * TOK_PER_CORE])
            lT = rps.tile([E, TOK_PER_CORE], f32, tag="lT", bufs=4, name="lT")
            for k in range(KD):
                nc.tensor.matmul(lT[:], lhsT=wg_sb[:, k, :], rhs=xsc[:, k, :],
                                 start=(k == 0), stop=(k == KD - 1))
            lsb = rpool.tile([E, TOK_PER_CORE], f32, tag="lsb", bufs=3,
                             name="lsb")
            nc.vector.tensor_copy(lsb[:], lT[:])
            lsbs[c] = lsb
            if c >= LAG:
                emit_transpose(c - LAG)
        for cc in range(NCH - LAG, NCH):
            emit_transpose(cc)
        nc.vector.tensor_add(l_all[:], l_all[:], bg_sb[:])
        nc.scalar.activation(exp_all[:], l_all[:],
                             mybir.ActivationFunctionType.Exp)
        # top-2 indicator via masked second-max (5 full-width ops)
        exp3 = exp_all[:].rearrange("p (t e) -> p t e", e=E)
        mx = rpool.tile([P, NT], f32, name="mx")
        nc.vector.reduce_max(mx[:], exp3, axis=mybir.AxisListType.X)
        eqm = rpool.tile([P, NE], f32, name="eqm")
        nc.vector.tensor_tensor(
            out=eqm[:].rearrange("p (t e) -> p t e", e=E), in0=exp3,
            in1=mx[:, :, None].to_broadcast([P, NT, E]),
            op=mybir.AluOpType.is_ge)
        nm = rpool.tile([P, NE], f32, name="nm")
        nc.vector.scalar_tensor_tensor(
            out=nm[:], in0=eqm[:], scalar=-BIGP, in1=exp_all[:],
            op0=mybir.AluOpType.mult, op1=mybir.AluOpType.add)
        m2 = rpool.tile([P, NT], f32, name="m2")
        nc.vector.reduce_max(m2[:], nm[:].rearrange("p (t e) -> p t e", e=E),
                             axis=mybir.AxisListType.X)
        nc.vector.tensor_tensor(
            out=ind_all[:].rearrange("p (t e) -> p t e", e=E), in0=exp3,
            in1=m2[:, :, None].to_broadcast([P, NT, E]),
            op=mybir.AluOpType.is_ge)
        # top-8 values/indices + softmax denom: needs only exp_all, so
        # Vector does it here (during the dispatch matmuls) instead of
        # competing with the idx one-hot ops later.
        for tt in range(NT):
            sl = slice(tt * E, (tt + 1) * E)
            nc.vector.max(out=m8_all[:, sl], in_=exp_all[:, sl])
            nc.vector.max_index(out=ei_all[:, sl], in_max=m8_all[:, sl],
                                in_values=exp_all[:, sl])
        s_all = rpool.tile([P, NT], f32, name="s_all")
        nc.vector.reduce_sum(s_all[:], exp_all[:].rearrange(
            "p (t e) -> p t e", e=E), axis=mybir.AxisListType.X)
        nc.vector.reciprocal(r_all[:], s_all[:])
        ei3 = ei_all[:].rearrange("p (t e) -> p t e", e=E)
        nc.vector.tensor_copy(e1f[:], ei3[:, :, 0])
        nc.vector.tensor_copy(e2f[:], ei3[:, :, 1])
        # W1 + W2 interleaved f-major: both are consumed in f order (W1 by
        # phase A, W2 by phase B ~15us later), so pairing the transfers
        # matches arrival to consumption.
        for f in range(NF):
            nc.sync.dma_start(w1_sb[f][:], w1d[f * P:(f + 1) * P, :])
            nc.sync.dma_start(w2_sb[f][:], w2d[f * P:(f + 1) * P, :])

    # ---------------- dispatch: cumsum positions ----------------
    with tc.tile_pool(name="disp_sb", bufs=1, named_scope="dispatch") as dpool, \
         tc.tile_pool(name="disp_ps", bufs=1, space="PSUM") as dps:
        warm_mms(dpool, dps, 24, "wmm1")
        # PSUM is bank-granular: pack the small intermediates into two
        # shared scratch banks addressed by column slices
        ps1 = dps.tile([P, 512], f32, name="ps1")
        ps2 = dps.tile([P, 512], f32, name="ps2")
        ptot_s = ps1[0:1, 0:NE]
        ppos_s = ps1[:, NE:2 * NE]
        t32_s = ps2[0:NT, 0:E]
        pofs_s = ps2[0:NT, E:2 * E]
        ofsT_s = ps2[0:E, 16:16 + NT]
        obc_s = ps2[:, 48:48 + NT]
        ppos2_s = ps2[:, 128:128 + NE]
        # per-(tile,expert) totals in one matmul
        nc.tensor.matmul(ptot_s, lhsT=ones_col[:], rhs=ind_all[:],
                         start=True, stop=True)
        tot_flat = dpool.tile([1, NE], f32, name="tot_flat")
        nc.vector.tensor_copy(tot_flat[:], ptot_s)
        # reshape [1, (t e)] -> [NT, E] with 8 strided transposes (no DMA)
        tf3 = tot_flat[:].rearrange("o (t e) -> o t e", e=E)
        for e in range(E):
            nc.tensor.transpose(t32_s[:, e:e + 1], tf3[:, :, e],
                                identf[:1, :1])
        tot32 = dpool.tile([NT, E], f32, name="tot32")
        nc.vector.tensor_copy(tot32[:], t32_s)
        # exclusive cumsum over tiles: strict-lower matmul
        nc.tensor.matmul(pofs_s, lhsT=l128_sb[:NT, :NT], rhs=tot32[:],
                         start=True, stop=True)
        ofs32 = dpool.tile([NT, E], f32, name="ofs32")
        nc.vector.tensor_copy(ofs32[:], pofs_s)
        # my expert's tile offsets broadcast over partitions
        nc.tensor.transpose(ofsT_s, ofs32[:], identf[:NT, :NT])
        ofsT = dpool.tile([E, NT], f32, name="ofsT")
        nc.vector.tensor_copy(ofsT[:], ofsT_s)
        nc.tensor.matmul(obc_s, lhsT=selE_sb[:], rhs=ofsT[:],
                         start=True, stop=True)
        # tile-local exclusive cumsum (no tile offsets yet)
        nc.tensor.matmul(ppos_s, lhsT=l128_sb[:], rhs=ind_all[:],
                         start=True, stop=True)
        nc.vector.tensor_copy(pos_all[:], ppos_s)

        # fast masked positions for my expert:
        #   posm = ind_e ? (local_me + ofs_me) : BIGP
        t1 = dpool.tile([P, NE], f32, name="t1")
        nc.vector.tensor_mul(t1[:], ind_all[:], sel_sb[:])
        t2 = dpool.tile([P, NE], f32, name="t2")
        nc.vector.tensor_mul(t2[:], t1[:], pos_all[:])
        r1 = dpool.tile([P, NT], f32, name="r1")
        nc.vector.reduce_sum(r1[:], t2[:].rearrange(
            "p (t e) -> p t e", e=E), axis=mybir.AxisListType.X)
        ind_e = dpool.tile([P, NT], f32, name="ind_e")
        nc.vector.reduce_sum(ind_e[:], t1[:].rearrange(
            "p (t e) -> p t e", e=E), axis=mybir.AxisListType.X)
        nc.vector.tensor_tensor(out=pos_m[:], in0=ind_e[:], in1=obc_s,
                                op=mybir.AluOpType.mult)
        nc.vector.tensor_add(pos_m[:], pos_m[:], r1[:])
        nc.vector.scalar_tensor_tensor(
            out=pos_m[:], in0=ind_e[:], scalar=-BIGP, in1=pos_m[:],
            op0=mybir.AluOpType.mult, op1=mybir.AluOpType.add)
        nc.vector.tensor_scalar_add(pos_m[:], pos_m[:], BIGP)

        # absorb the dynamic-queue cold start before the real gathers
        wrg2 = dpool.tile([P, 2, D], bf16, name="wrg2")
        for wi in range(2):
            nc.gpsimd.indirect_dma_start(
                out=wrg2[:, wi, :], out_offset=None, in_=xbf[:],
                in_offset=IndirectOffsetOnAxis(ap=wrmi[:], axis=0))
        # finish full pos_all for the combine: flatten ofs32 to one
        # partition with 8 strided transposes (no DMA!), then broadcast
        # to all partitions via a rank-1 matmul
        ofl_s = ps1[0:1, 0:NE]  # reuses ptot's bank region (ptot done)
        for e in range(E):
            nc.tensor.transpose(
                ofl_s.rearrange("o (t e) -> o t e", e=E)[:, :, e],
                ofs32[:, e:e + 1], identf[:NT, :NT])
        ofs_flat = dpool.tile([1, NE], f32, name="ofs_flat")
        nc.vector.tensor_copy(ofs_flat[:], ofl_s)
        nc.tensor.matmul(ppos2_s, lhsT=ones_row[:], rhs=ofs_flat[:],
                         start=True, stop=True)
        nc.vector.tensor_tensor(out=pos_all[:], in0=pos_all[:],
                                in1=ppos2_s, op=mybir.AluOpType.add)

    # ---------------- FFN + dispatch idx/gather/transpose pipeline ----------
    with tc.tile_pool(name="ffn_sb", bufs=1, named_scope="ffn") as fpool, \
         tc.tile_pool(name="ffn_ps", bufs=1, space="PSUM") as fps, \
         tc.tile_pool(name="comb_sb", bufs=1, named_scope="combine") as cpool:

        def emit_idx_steps(g):
            """Slot->token map for group g as 32 interleavable steps plus
            a finisher; steps are woven into a phase-A matmul stream."""
            sz, base, tg = gsz[g], gbase[g], gtiles[g]
            acc = fps.tile([2, 512], f32, tag="accx", bufs=1, name="accx")

            def step(tt):
                Pt = fpool.tile([P, 384], bf16, tag="Pt", bufs=2, name="Pt")
                nc.vector.tensor_scalar(
                    Pt[:, :sz], iotaC[:, base:base + sz],
                    pos_m[:, tt:tt + 1], None,
                    op0=mybir.AluOpType.is_equal)
                nc.tensor.matmul(acc[:, :sz], lhsT=pv2[:, tt, :],
                                 rhs=Pt[:, :sz],
                                 start=(tt == 0), stop=(tt == NT - 1))

            return [lambda tt=tt: step(tt) for tt in range(NT)], \
                lambda: idx_finish(g, acc)

        def idx_finish(g, acc):
            sz, base, tg = gsz[g], gbase[g], gtiles[g]
            idx2_sb = fpool.tile([2, 384], f32, tag="idx2", bufs=1,
                                 name="idx2_sb")
            nc.vector.tensor_copy(idx2_sb[:, :sz], acc[:, :sz])
            pti_ps = fps.tile([P, 8], f32, tag="pti", bufs=1, name="pti")
            for t in range(tg):
                nc.tensor.transpose(pti_ps[:, 2 * t:2 * t + 2],
                                    idx2_sb[:, t * P:(t + 1) * P],
                                    identf[:2, :2])
            pti_sb = fpool.tile([P, 4, 2], f32, tag="pti_sb", bufs=2,
                                name="pti_sb")
            nc.vector.tensor_copy(
                pti_sb[:, :tg, :].rearrange("p t o -> p (t o)"),
                pti_ps[:, :2 * tg])
            idx_f = fpool.tile([P, 4], f32, tag="idx_f", bufs=2,
                               name="idx_f")
            nc.vector.scalar_tensor_tensor(
                out=idx_f[:, :tg], in0=pti_sb[:, :tg, 1], scalar=float(P),
                in1=pti_sb[:, :tg, 0], op0=mybir.AluOpType.mult,
                op1=mybir.AluOpType.add)
            nc.vector.tensor_copy(idx_i[:, base // P:base // P + tg],
                                  idx_f[:, :tg])
            xga = []
            for t in range(tg):
                s = base // P + t
                xg = fpool.tile([P, D], bf16, tag="xga", bufs=3, name="xga")
                nc.gpsimd.indirect_dma_start(
                    out=xg[:], out_offset=None, in_=xbf[:],
                    in_offset=IndirectOffsetOnAxis(ap=idx_i[:, s:s + 1],
                                                   axis=0),
                )
                xga.append(xg)
            xga_pend[g] = xga

        def emit_T(g, xga):
            """Transpose gathered slot tiles into a per-group xgT (TensorE)."""
            tg = gtiles[g]
            xgt = fpool.tile([P, KD, 384], bf16, tag="xgT", bufs=1,
                             name="xgT")
            for t in range(tg):
                for d in range(KD):
                    pt = fps.tile([P, P], bf16, tag="ptT", bufs=2, name="ptT")
                    nc.tensor.transpose(pt[:], xga[t][:, d * P:(d + 1) * P],
                                        ident[:])
                    if d % 2 == 0:
                        nc.vector.tensor_copy(
                            xgt[:, d, t * P:(t + 1) * P], pt[:])
                    else:
                        nc.scalar.activation(
                            xgt[:, d, t * P:(t + 1) * P], pt[:],
                            mybir.ActivationFunctionType.Copy)
            return xgt

        def emit_A(g, xgt, steps=None, finish=None):
            """Layer-1 for group g: all f tiles -> ReLU -> hbuf list.
            Interleaves the NEXT idx build's one-hot matmuls (2 per f tile
            from f=8) into the stream so they never pace a group boundary.
            """
            sz = gsz[g]
            hb = []
            for f in range(NF):
                ph = fps.tile([P, 384], f32, tag="ph", bufs=2, name="ph")
                for k in range(KD):
                    nc.tensor.matmul(
                        ph[:, :sz], lhsT=w1_sb[f][:, k * P:(k + 1) * P],
                        rhs=xgt[:, k, :sz],
                        start=(k == 0), stop=(k == KD - 1))
                h = fpool.tile([P, 384], bf16, tag="hb", bufs=32, name="hb")
                nc.scalar.activation(h[:, :sz], ph[:, :sz],
                                     mybir.ActivationFunctionType.Relu,
                                     bias=b1_sb[:, f:f + 1], scale=1.0)
                hb.append(h)
                if steps is not None and f >= 8:
                    for _ in range(2):
                        if steps:
                            steps.pop(0)()
                    if not steps and finish is not None:
                        finish()
                        finish = None
            if steps:
                while steps:
                    steps.pop(0)()
            if finish is not None:
                finish()
            return hb

        def emit_B(g, hb):
            """Layer-2 for group g (output-stationary), stage + AllGather."""
            sz, base, tg = gsz[g], gbase[g], gtiles[g]
            for t in range(tg):
                ysb = fpool.tile([P, D], bf16, tag="ysb", bufs=2, name="ysb")
                for n in range(2):
                    py = fps.tile([P, 512], f32, tag="py", bufs=2, name="py")
                    for f in range(NF):
                        nc.tensor.matmul(
                            py[:], lhsT=hb[f][:, t * P:(t + 1) * P],
                            rhs=w2_sb[f][:, n * 512:(n + 1) * 512],
                            start=(f == 0), stop=False)
                    nc.tensor.matmul(
                        py[:], lhsT=ones_rb[:],
                        rhs=b2_sb[:, n * 512:(n + 1) * 512],
                        start=False, stop=True)
                    nc.scalar.activation(
                        ysb[:, n * 512:(n + 1) * 512], py[:],
                        mybir.ActivationFunctionType.Copy)
                nc.sync.dma_start(y_dram[g][t * P:(t + 1) * P, :], ysb[:])
            # ship this chunk while the next group computes
            nc.gpsimd.collective_compute(
                "AllGather", mybir.AluOpType.bypass,
                replica_groups=[list(range(N_CORES))],
                ins=[y_dram[g][:].opt()],
                outs=[y_all[g][:].opt()],
            )

        ytb = cpool.tile([P, OWN_TILES, D], bf16, name="ytb")
        nc.vector.memset(ytb[:], 0.0)

        def emit_combine(g):
            """Gather + gate-weighted accumulate for chunk g."""
            for q in range(2):
                yt = ytb
                for j in range(OWN_TILES):
                    nc.gpsimd.indirect_dma_start(
                        out=yt[:, j, :], out_offset=None, in_=y_all[g][:],
                        in_offset=IndirectOffsetOnAxis(
                            ap=redi_sb[:, g,
                                       q * OWN_TILES + j:
                                       q * OWN_TILES + j + 1],
                            axis=0),
                        bounds_check=N_CORES * gsz[g] - 1,
                        oob_is_err=False)
                for j in range(OWN_TILES):
                    k = 2 * g + q
                    w = red_sb[:, j, NSEL + k:NSEL + k + 1]
                    if g == 0 and q == 0:
                        nc.vector.tensor_scalar(
                            ot[j][:], yt[:, j, :], w, None,
                            op0=mybir.AluOpType.mult)
                    else:
                        nc.vector.scalar_tensor_tensor(
                            out=ot[j][:], in0=yt[:, j, :],
                            scalar=w, in1=ot[j][:],
                            op0=mybir.AluOpType.mult,
                            op1=mybir.AluOpType.add)
                    if g == NG - 1 and q == 1:
                        nc.sync.dma_start(out[j * P:(j + 1) * P, :], ot[j][:])

        def emit_select():
            """Combine selection stack (Vector; overlaps A(g0))."""
            ioz = cpool.tile([P, NE], mybir.dt.int16, name="ioz")
            nc.gpsimd.iota(ioz[:].rearrange("p (t e) -> p t e", e=E),
                           pattern=[[0, NT], [1, E]], base=0,
                           channel_multiplier=0)
            iof = cpool.tile([P, NE], bf16, name="iof")
            nc.vector.tensor_copy(iof[:], ioz[:])
            m83 = m8_all[:].rearrange("p (t e) -> p t e", e=E)
            Ssel = cpool.tile([P, 2 * NSEL, NT], f32, name="Ssel")
            for q, ef in ((0, e1f), (1, e2f)):
                oh = cpool.tile([P, NE], f32, tag="oh", bufs=1, name="oh")
                nc.vector.tensor_tensor(
                    out=oh[:].rearrange("p (t e) -> p t e", e=E),
                    in0=iof[:].rearrange("p (t e) -> p t e", e=E),
                    in1=ef[:, :, None].to_broadcast([P, NT, E]),
                    op=mybir.AluOpType.is_equal)
                nc.vector.tensor_mul(oh[:], oh[:], pos_all[:])
                slot = cpool.tile([P, NT], f32, tag="slot", bufs=1,
                                  name="slot")
                nc.vector.reduce_sum(slot[:], oh[:].rearrange(
                    "p (t e) -> p t e", e=E), axis=mybir.AxisListType.X)
                gch = cpool.tile([P, NT], f32, tag="gch", bufs=1, name="gch")
                nc.vector.tensor_scalar(gch[:], slot[:], float(gbase[1]),
                                        None, op0=mybir.AluOpType.is_ge)
                for gg in range(2, NG):
                    t2 = cpool.tile([P, NT], f32, tag="t2", name="t2")
                    nc.vector.tensor_scalar(t2[:], slot[:],
                                            float(gbase[gg]), None,
                                            op0=mybir.AluOpType.is_ge)
                    nc.vector.tensor_add(gch[:], gch[:], t2[:])
                gv = cpool.tile([P, NT], f32, tag="gv", bufs=1, name="gv")
                nc.vector.tensor_tensor(out=gv[:], in0=m83[:, :, q],
                                        in1=r_all[:],
                                        op=mybir.AluOpType.mult)
                for gg in range(NG):
                    k = 2 * gg + q
                    eq = cpool.tile([P, NT], f32, tag="eq", name="eq")
                    nc.vector.tensor_scalar(eq[:], gch[:], float(gg), None,
                                            op0=mybir.AluOpType.is_equal)
                    # offset within chunk gg: e*gsz + slot - gbase
                    base_t = cpool.tile([P, NT], f32, tag="bt", name="bt")
                    nc.vector.scalar_tensor_tensor(
                        out=base_t[:], in0=ef[:], scalar=float(gsz[gg]),
                        in1=slot[:], op0=mybir.AluOpType.mult,
                        op1=mybir.AluOpType.add)
                    # masked rows get a huge offset: the combine gather
                    # bounds-checks and SKIPS them (no wasted DMA bytes)
                    nc.vector.tensor_scalar_add(
                        Ssel[:, k, :], base_t[:],
                        float(-gbase[gg] - 1000000.0))
                    nc.vector.tensor_mul(Ssel[:, k, :], Ssel[:, k, :], eq[:])
                    nc.vector.tensor_scalar_add(Ssel[:, k, :], Ssel[:, k, :],
                                                1000000.0)
                    nc.vector.tensor_mul(Ssel[:, NSEL + k, :], eq[:], gv[:])
            for j in range(OWN_TILES):
                own = ownm_sb[:, j * NT:(j + 1) * NT]
                for h2 in range(2):
                    tmpS = cpool.tile([P, NSEL, NT], f32, tag="tmpS",
                                      bufs=1, name="tmpS")
                    nc.vector.tensor_tensor(
                        out=tmpS[:], in0=Ssel[:, h2 * NSEL:(h2 + 1) * NSEL, :],
                        in1=own[:, None, :].to_broadcast([P, NSEL, NT]),
                        op=mybir.AluOpType.mult)
                    nc.vector.reduce_sum(
                        red_sb[:, j, h2 * NSEL:(h2 + 1) * NSEL], tmpS[:],
                        axis=mybir.AxisListType.X)
            # pack gather offsets as [g, (q j)] int planes
            for gg in range(NG):
                for q in range(2):
                    nc.vector.tensor_copy(
                        redi_sb[:, gg, q * OWN_TILES:(q + 1) * OWN_TILES],
                        red_sb[:, :, 2 * gg + q])

        # -------- pipeline --------
        # Engine-queue ordering follows emission order: combine STTs (which
        # wait on an AllGather) are emitted after the NEXT group's B so no
        # AG trigger queues behind them, and each idx build's one-hot
        # matmuls are interleaved into the PREVIOUS group's phase-A stream.
        xga_pend = {}
        steps0, fin0 = emit_idx_steps(0)
        for s in steps0:
            s()
        fin0()
        xgt = emit_T(0, xga_pend.pop(0))
        if NG > 1:
            steps, fin = emit_idx_steps(1)
        else:
            steps, fin = None, None
        hb = emit_A(0, xgt, steps, fin)
        emit_select()
        for g in range(NG):
            emit_B(g, hb)
            if g >= 1:
                emit_combine(g - 1)
            if g + 1 < NG:
                xgt = emit_T(g + 1, xga_pend.pop(g + 1))
                if g + 2 < NG:
                    steps, fin = emit_idx_steps(g + 2)
                else:
                    steps, fin = None, None
                hb = emit_A(g + 1, xgt, steps, fin)
        emit_combine(NG - 1)

    persist.release()
    dram.release()


def _host_prep(x, Wg, bg, W1, b1, W2, b2):
    xf = np.ascontiguousarray(x.reshape(T, D).astype(np.float32))
    xT = np.ascontiguousarray(xf.T)
    xbf = xf.astype(ml_dtypes.bfloat16)
    bgb = np.tile(bg.astype(np.float32), NT)[None, :].repeat(P, 0)
    bgb = np.ascontiguousarray(bgb)
    l128 = np.triu(np.ones((P, P), np.float32), 1)  # [t', t] = 1 if t' < t
    in_maps = []
    for c in range(N_CORES):
        sel = np.zeros(E, np.float32)
        sel[c] = 1.0
        sel256 = np.ascontiguousarray(np.tile(sel, NT)[None, :].repeat(P, 0))
        ownm = np.zeros((P, OWN_TILES, NT), np.float32)
        for j in range(OWN_TILES):
            ownm[:, j, OWN_TILES * c + j] = 1.0
        in_maps.append({
            "xTf": xT,
            "xbf": xbf,
            "w1d": np.ascontiguousarray(
                W1[c].astype(ml_dtypes.bfloat16)
                .reshape(KD, P, NF, P).transpose(2, 1, 0, 3)
                .reshape(NF * P, KD * P)),
            "w2d": np.ascontiguousarray(W2[c].astype(ml_dtypes.bfloat16)),
            "wgd": np.ascontiguousarray(Wg.astype(np.float32)),
            "bgb": bgb,
            "b1pm": np.ascontiguousarray(
                b1[c].astype(np.float32).reshape(NF, P).T),
            "b2bd": np.ascontiguousarray(
                b2[c].astype(ml_dtypes.bfloat16)[None, :]),
            "sel256": sel256,
            "selEd": np.ascontiguousarray(
                sel[:, None].repeat(P, 1).astype(np.float32)),
            "l128d": l128,
            "ownmd": np.ascontiguousarray(ownm.reshape(P, OWN_TILES * NT)),
        })
    return in_maps


def _capacity_tiles(x, Wg, bg):
    xf = x.reshape(T, D).astype(np.float32)
    logits = xf @ Wg.astype(np.float32) + bg.astype(np.float32)
    part = np.partition(logits, E - 2, axis=-1)
    m2 = part[:, E - 2:E - 1]
    counts = (logits >= m2).sum(0)
    return int(np.ceil((counts.max() + 16) / P))


LAST_RESULT = None


def kernel(x, Wg, bg, W1, b1, W2, b2):
    global LAST_RESULT
    from concourse.bass_utils import run_bass_kernel_spmd

    x = np.asarray(x)
    ST = _capacity_tiles(x, np.asarray(Wg), np.asarray(bg))
    if ST not in _cache:
        _cache[ST] = build_module(ST)
    nc = _cache[ST]
    in_maps = _host_prep(x, np.asarray(Wg), np.asarray(bg), np.asarray(W1),
                         np.asarray(b1), np.asarray(W2), np.asarray(b2))
    trace = bool(os.environ.get("BASS_TRACE"))
    if trace:
        _setup_axon_profile_hook()
    res = run_bass_kernel_spmd(nc, in_maps, core_ids=list(range(N_CORES)),
                               trace=trace)
    LAST_RESULT = res
    out = np.empty((T, D), np.float32)
    for c in range(N_CORES):
        out[c * TOK_PER_CORE:(c + 1) * TOK_PER_CORE] = \
            res.results[c]["out"].astype(np.float32)
    return out.reshape(B, S, D)


def _setup_axon_profile_hook():
    """Provide antenv.axon_hooks (missing in this image) so trace=True works."""
    import types
    try:
        import antenv
        if "antenv.axon_hooks" not in sys.modules:
            hooks = types.ModuleType("antenv.axon_hooks")
            hooks._hook = None
            hooks.set_axon_ntff_profile_hook = \
                lambda h: setattr(hooks, "_hook", h)
            hooks.get_axon_ntff_profile_hook = lambda: hooks._hook
            sys.modules["antenv.axon_hooks"] = hooks
            antenv.axon_hooks = hooks
            from trn_agent_boot.trn_boot import _ntff_profile_via_ctypes
            hooks.set_axon_ntff_profile_hook(
                _ntff_profile_via_ctypes("/opt/axon/libaxon_pjrt.so"))
    except Exception as e:  # profiling is best-effort
        print(f"profile hook setup failed: {e}", file=sys.stderr)
